# revision 16
# baseline (speedup 1.0000x reference)
"""Trainium2 Bass kernel for nn_Loss_for_localization (YOLO-style loss).

Strategy: pure data parallel over the batch dim (8 images per core), with
host-side dtype compression to cut HBM traffic — the kernel is DMA-bound, so
bytes-on-wire is the whole game:

  objects  -> fp16 (clamped to <=1-2^-11 so log1p(-p) can't hit -inf)
  gt mask  -> uint8 (exact 0/1)
  locs/gt coords -> fp16
  scores/label   -> fp32 (tiny)

Per core the host packs everything into ONE contiguous uint8 tensor
[A=9, 128, 19456]: for each anchor iteration, partition p = b_local*16 + s
holds bytes [p16(2048) | mask(1024) | l16(8192) | g16(8192)], i.e. 1024 cells
of image b's plane (sixteenth s). One dma_start per iteration moves 2.49 MB
with 19456-byte contiguous runs per partition; the HWDGE round-robins
partitions across all SDMA engines. 22.4 MB/core total vs 47.2 MB for fp32.

On-chip per iteration (all elementwise tiles [128, 1024/4096], fp16 so the
DVE runs 2x):
  ACT: lg0 = Ln(1-p) (accum_out -> sum), lg1 = Ln(p)
  DVE: mf = copy(mask u8 -> f16); two stt ops (m * -lg0, m * -lg1, accum);
       dd = l - g  [128,4096]; per channel: md_c = m*d_c, then
       stt md_c*md_c with accum (= m*d^2 since m in {0,1}).
The -100 BCE clamp never binds: fp32-uniform p >= 2^-24 -> |log| <= 16.64,
and the fp16 clamp keeps 1-p >= 2^-11.

Scores cross-entropy + epilogue (weighted combine, cross-partition
ones-matmul reduce) as before. Host sums the 8 partial scalars / B.

This file also carries a workaround for the container's walrus build, which
accepts at most ONE sync-wait and ONE sem-update per instruction: a BIR-JSON
post-pass (hooked into Bass.to_json_bytes) moves excess waits/updates onto
EventSemaphore carrier instructions on the same engine queue.
"""

import sys

sys.path.insert(0, "/opt/trn_rl_repo")

from contextlib import ExitStack

import numpy as np
import orjson

import concourse.bass as bass
import concourse.mybir as mybir
import concourse.tile as tile
from concourse.bass_utils import run_bass_kernel_spmd

f32 = mybir.dt.float32
f16 = mybir.dt.float16
u8 = mybir.dt.uint8
ALU = mybir.AluOpType
ACTF = mybir.ActivationFunctionType
AX = mybir.AxisListType

N_CORES = 8
B, A, H, W, C = 64, 9, 128, 128, 1000
BL = B // N_CORES  # 8 local batches per core
S = 16             # sixteenths of a plane; partition = b_local*16 + s
F = (H * W) // S   # 1024 cells per partition per plane

# packed byte layout per (iter, partition)
OFF_P = 0
OFF_M = OFF_P + 2 * F          # 2048
OFF_L = OFF_M + 2 * F          # 4096
OFF_G = OFF_L + 2 * 4 * F      # 12288
NB = OFF_G + 2 * 4 * F         # 20480

MAXF16_BELOW1 = np.float16(np.nextafter(np.float16(1.0), np.float16(0.0)))

# ---------------------------------------------------------------------------
# walrus <=1 sync-wait / <=1 sem-update per instruction workaround
# ---------------------------------------------------------------------------
_split_counter = [0]


def _carrier(engine, debug, sync_info):
    _split_counter[0] += 1
    return {
        "opcode": "EventSemaphore",
        "engine": engine,
        "ins": [],
        "outs": [],
        "name": f"splitsync-{_split_counter[0]}",
        "debug": debug,
        "sync_info": sync_info,
    }


def _split_excess_sync_json(bir: bytes) -> bytes:
    m = orjson.loads(bir)
    changed = False
    for fn in m.get("functions", []):
        for bb in fn.get("blocks", []):
            instrs = bb.get("instructions")
            if not instrs:
                continue
            out = []
            for ins in instrs:
                si = ins.get("sync_info")
                followers = []
                if si:
                    waits = si.get("on_wait") or []
                    if len(waits) > 1:
                        changed = True
                        for w in waits[:-1]:
                            out.append(
                                _carrier(ins["engine"], ins.get("debug"),
                                         {"on_wait": [w], "on_update": []})
                            )
                        si["on_wait"] = waits[-1:]
                    ups = si.get("on_update") or []
                    if len(ups) > 1:
                        assert ins.get("opcode") != "DMACopy", (
                            "cannot split sem updates off an async DMACopy"
                        )
                        changed = True
                        for u in ups[1:]:
                            followers.append(
                                _carrier(ins["engine"], ins.get("debug"),
                                         {"on_wait": [], "on_update": [u]})
                            )
                        si["on_update"] = ups[:1]
                out.append(ins)
                out.extend(followers)
            bb["instructions"] = out
    if not changed:
        return bir
    return orjson.dumps(m)


if not getattr(bass.Bass, "_sync_split_patched", False):
    _orig_to_json_bytes = bass.Bass.to_json_bytes

    def _patched_to_json_bytes(self):
        return _split_excess_sync_json(_orig_to_json_bytes(self))

    bass.Bass.to_json_bytes = _patched_to_json_bytes
    bass.Bass._sync_split_patched = True


# ---------------------------------------------------------------------------
# program builder
# ---------------------------------------------------------------------------
def _build_program():
    nc = bass.Bass()
    packed_l = nc.declare_dram_parameter("packed", [A, 128, NB], u8, isOutput=False)
    scores_l = nc.declare_dram_parameter("scores", [BL, C], f32, isOutput=False)
    label_l = nc.declare_dram_parameter("label", [BL, 1], f32, isOutput=False)
    out_d = nc.declare_dram_parameter("out", [1, 1], f32, isOutput=True)

    with tile.TileContext(nc) as tc, ExitStack() as octx:
        # long-lived accumulators
        fin = octx.enter_context(tc.tile_pool(name="fin", bufs=1))
        acc_lg0 = fin.tile([128, A], f32)
        acc_msq = fin.tile([128, 2 * A], f32)
        nc.vector.memset(acc_lg0[:], 0.0)
        nc.vector.memset(acc_msq[:], 0.0)


        sct = fin.tile([BL, C], f32)
        labt = fin.tile([BL, 1], f32)

        # ones column for PE column-sum reductions
        ones = fin.tile([128, 1], f16)
        nc.vector.memset(ones[:], 1.0)
        # PSUM accumulators for the masked-log column sums (accumulate
        # across all iterations; read once in the epilogue)
        psacc = octx.enter_context(tc.tile_pool(name="psacc", bufs=1, space="PSUM"))
        ps_p0 = psacc.tile([1, 512], f32)
        ps_p1 = psacc.tile([1, 512], f32)

        # ---- main streaming loop ----
        # ACT's Square runs one iteration behind (software pipelining) so the
        # intra-iteration ACT->DVE->ACT chain never serializes; DVE issues its
        # ACT-independent ops (mask copy, subtract) ahead of the products so
        # its queue head never stalls waiting on Ln.
        with ExitStack() as ctx:
            inp = ctx.enter_context(tc.tile_pool(name="inp", bufs=5))
            mid = ctx.enter_context(tc.tile_pool(name="mid", bufs=2))

            def bcast4(ap):
                # [128, F] AP -> [128, 4, F] with a stride-0 channel dim
                return bass.AP(ap.tensor, ap.offset,
                               [list(ap.ap[0]), [0, 4], list(ap.ap[1])])

            md_prev = None
            for a in range(A):
                big = inp.tile([128, NB], u8, tag="big")
                # small (objects+mask) first so ACT/DVE work on iteration 0
                # starts well before the big coords transfer lands
                nc.sync.dma_start(big[:, :OFF_L], packed_l[a][:, :OFF_L])
                nc.sync.dma_start(big[:, OFF_L:], packed_l[a][:, OFF_L:])

                p16 = big[:, OFF_P:OFF_M].bitcast(f16)   # [128,1024]
                mfv = big[:, OFF_M:OFF_L].bitcast(f16)   # [128,1024] mask
                l16 = big[:, OFF_L:OFF_G].bitcast(f16)   # [128,4096]
                g16 = big[:, OFF_G:NB].bitcast(f16)      # [128,4096]

                lg0 = mid.tile([128, F], f16, tag="lg0")
                nc.scalar.activation(lg0[:], p16, ACTF.Ln, bias=1.0, scale=-1.0,
                                     accum_out=acc_lg0[:, a:a + 1])
                lg1 = mid.tile([128, F], f16, tag="lg1")
                nc.scalar.activation(lg1[:], p16, ACTF.Ln)
                if md_prev is not None:
                    sq = mid.tile([128, 4 * F], f16, tag="sq", bufs=1)
                    for h in range(2):
                        hs = slice(2 * F * h, 2 * F * (h + 1))
                        nc.scalar.activation(
                            sq[:, hs], md_prev[:, hs], ACTF.Square,
                            accum_out=acc_msq[:, 2 * (a - 1) + h:2 * (a - 1) + h + 1])

                dd = mid.tile([128, 4 * F], f16, tag="dd")
                nc.vector.tensor_tensor(out=dd[:], in0=l16, in1=g16,
                                        op=ALU.subtract)

                # md = m * d per channel (2x tt); m^2 = m, so Square(md)
                # with accum_out on ACT gives sum(m * d^2) for free.
                # Issued before pr0/pr1 so the DVE queue head never waits
                # on ACT's Ln outputs.
                md = mid.tile([128, 4 * F], f16, tag="md")
                for c in range(4):
                    nc.vector.tensor_tensor(
                        out=md[:, c * F:(c + 1) * F],
                        in0=dd[:, c * F:(c + 1) * F], in1=mfv, op=ALU.mult)
                md_prev = md

                # masked log sums: tt product (2x), then PE column-sum
                # accumulated into ps_p0/ps_p1 across all iterations
                pr0 = mid.tile([128, F], f16, tag="pr0")
                nc.vector.tensor_tensor(out=pr0[:], in0=mfv, in1=lg0[:],
                                        op=ALU.mult)
                pr1 = mid.tile([128, F], f16, tag="pr1")
                nc.vector.tensor_tensor(out=pr1[:], in0=mfv, in1=lg1[:],
                                        op=ALU.mult)
                for h in range(2):
                    sl = slice(512 * h, 512 * (h + 1))
                    nc.tensor.matmul(ps_p0[:], ones[:], pr0[:, sl],
                                     start=(a == 0 and h == 0),
                                     stop=(a == A - 1 and h == 1))
                    nc.tensor.matmul(ps_p1[:], ones[:], pr1[:, sl],
                                     start=(a == 0 and h == 0),
                                     stop=(a == A - 1 and h == 1))

            sq = mid.tile([128, 4 * F], f16, tag="sq", bufs=1)
            for h in range(2):
                hs = slice(2 * F * h, 2 * F * (h + 1))
                nc.scalar.activation(
                    sq[:, hs], md_prev[:, hs], ACTF.Square,
                    accum_out=acc_msq[:, 2 * (A - 1) + h:2 * (A - 1) + h + 1])

        # ---- image-classification cross entropy ----
        img = fin.tile([BL, 1], f32)
        sco_ctx = ExitStack()
        sco = sco_ctx.enter_context(tc.tile_pool(name="sco", bufs=1))
        nc.sync.dma_start(sct[:], scores_l[:])
        nc.sync.dma_start(labt[:], label_l[:])
        mx = sco.tile([BL, 1], f32)
        nc.vector.tensor_reduce(out=mx[:], in_=sct[:], axis=AX.X, op=ALU.max)
        nmx = sco.tile([BL, 1], f32)
        nc.vector.tensor_scalar(out=nmx[:], in0=mx[:], scalar1=-1.0,
                                scalar2=None, op0=ALU.mult)
        et = sco.tile([BL, C], f32)
        se = sco.tile([BL, 1], f32)
        nc.scalar.activation(et[:], sct[:], ACTF.Exp, bias=nmx[:], scale=1.0,
                             accum_out=se[:])
        lse0 = sco.tile([BL, 1], f32)
        nc.scalar.activation(lse0[:], se[:], ACTF.Ln)
        lse = sco.tile([BL, 1], f32)
        nc.vector.tensor_tensor(out=lse[:], in0=lse0[:], in1=mx[:], op=ALU.add)

        io = sco.tile([BL, C], mybir.dt.int32)
        nc.gpsimd.iota(io[:], pattern=[[1, C]], base=0, channel_multiplier=0)
        iof = sco.tile([BL, C], f32)
        nc.vector.tensor_copy(iof[:], io[:])
        oh = sco.tile([BL, C], f32)
        nc.vector.tensor_scalar(out=oh[:], in0=iof[:], scalar1=labt[:],
                                scalar2=None, op0=ALU.is_equal)
        jk2 = sco.tile([BL, C], f32)
        pk = sco.tile([BL, 1], f32)
        nc.vector.scalar_tensor_tensor(out=jk2[:], in0=oh[:], scalar=1.0,
                                       in1=sct[:], op0=ALU.mult, op1=ALU.mult,
                                       accum_out=pk[:])
        nc.vector.scalar_tensor_tensor(out=img[:], in0=pk[:], scalar=-1.0,
                                       in1=lse[:], op0=ALU.mult, op1=ALU.add)
        sco_ctx.close()

        # ---- per-partition reductions & weighted combine ----
        # total = img_sum - 0.5*sum(acc_lg0) + 0.5*sum(ps_p0) - sum(ps_p1)
        #         + 5*sum(acc_msq)
        with ExitStack() as ectx:
            epi = ectx.enter_context(tc.tile_pool(name="epi", bufs=1))
            psum = ectx.enter_context(tc.tile_pool(name="psum", bufs=1, space="PSUM"))

            s_lg0 = epi.tile([128, 1], f32)
            s_msq = epi.tile([128, 1], f32)
            nc.vector.tensor_reduce(out=s_lg0[:], in_=acc_lg0[:], axis=AX.X, op=ALU.add)
            nc.vector.tensor_reduce(out=s_msq[:], in_=acc_msq[:], axis=AX.X, op=ALU.add)

            tB = epi.tile([128, 1], f32)
            nc.vector.tensor_scalar(out=tB[:], in0=s_lg0[:], scalar1=-0.5,
                                    scalar2=None, op0=ALU.mult)
            tvec = epi.tile([128, 1], f32)
            nc.vector.scalar_tensor_tensor(out=tvec[:], in0=s_msq[:], scalar=5.0,
                                           in1=tB[:], op0=ALU.mult, op1=ALU.add)

            # ---- cross-partition reduce via ones-matmul into PSUM ----
            ones32 = epi.tile([128, 1], f32)
            nc.vector.memset(ones32[:], 1.0)
            acc = psum.tile([1, 1], f32)
            nc.tensor.matmul(acc[:], ones32[:], tvec[:], start=True, stop=False)
            nc.tensor.matmul(acc[:], ones32[:BL, :], img[:], start=False, stop=True)

            # fold in the PSUM column-sum accumulators
            jk0 = epi.tile([1, 512], f32)
            s0 = epi.tile([1, 1], f32)
            nc.scalar.activation(jk0[:], ps_p0[:], ACTF.Copy, accum_out=s0[:])
            jk1 = epi.tile([1, 512], f32)
            s1 = epi.tile([1, 1], f32)
            nc.scalar.activation(jk1[:], ps_p1[:], ACTF.Copy, accum_out=s1[:])

            accs = epi.tile([1, 1], f32)
            nc.scalar.copy(out=accs[:], in_=acc[:])
            f1 = epi.tile([1, 1], f32)
            nc.vector.scalar_tensor_tensor(out=f1[:], in0=s0[:], scalar=0.5,
                                           in1=accs[:], op0=ALU.mult, op1=ALU.add)
            res = epi.tile([1, 1], f32)
            nc.vector.scalar_tensor_tensor(out=res[:], in0=s1[:], scalar=-1.0,
                                           in1=f1[:], op0=ALU.mult, op1=ALU.add)
            nc.sync.dma_start(out_d[:], res[:])

    return nc


_program_cache = {}


def _get_program():
    if "nc" not in _program_cache:
        _program_cache["nc"] = _build_program()
    return _program_cache["nc"]


def _pack_core(objects, locs, gt):
    """Per-core [BL,...] inputs -> packed u8 [A, 128, NB].

    Partition p = b*16 + s; each partition's F cells are the s-th sixteenth
    of image b's (anchor-a) plane, flattened h*W+w order.
    """
    p16 = np.minimum(objects.astype(np.float16), MAXF16_BELOW1)
    p16 = p16.reshape(BL, A, S, F).transpose(1, 0, 2, 3)       # [A,BL,S,F]
    p16 = np.ascontiguousarray(p16).reshape(A, 128, F)
    m16 = gt[:, :, 0].astype(np.float16)
    m16 = m16.reshape(BL, A, S, F).transpose(1, 0, 2, 3)
    m16 = np.ascontiguousarray(m16).reshape(A, 128, F)
    l16 = locs.astype(np.float16).reshape(BL, A, 4, S, F).transpose(1, 0, 3, 2, 4)
    l16 = np.ascontiguousarray(l16).reshape(A, 128, 4 * F)     # [A,128,4096]
    g16 = gt[:, :, 1:5].astype(np.float16).reshape(BL, A, 4, S, F)
    g16 = np.ascontiguousarray(g16.transpose(1, 0, 3, 2, 4)).reshape(A, 128, 4 * F)
    return np.concatenate(
        [p16.view(np.uint8), m16.view(np.uint8), l16.view(np.uint8), g16.view(np.uint8)], axis=2)


def kernel(objects, scores, locs, label, gt, _trace=False, _trace_kwargs=None):
    objects = np.asarray(objects, dtype=np.float32)
    scores = np.ascontiguousarray(np.asarray(scores, dtype=np.float32))
    locs = np.asarray(locs, dtype=np.float32)
    gt = np.asarray(gt, dtype=np.float32)
    labf = np.asarray(label).astype(np.float32).reshape(B, 1)

    nc = _get_program()
    in_maps = []
    for i in range(N_CORES):
        sl = slice(i * BL, (i + 1) * BL)
        in_maps.append({
            "packed": _pack_core(objects[sl], locs[sl], gt[sl]),
            "scores": scores[sl],
            "label": np.ascontiguousarray(labf[sl]),
        })

    kw = {}
    if _trace:
        kw["trace"] = True
        kw.update(_trace_kwargs or {})
    res = run_bass_kernel_spmd(nc, in_maps, list(range(N_CORES)), **kw)
    partials = [float(res.results[i]["out"][0, 0]) for i in range(N_CORES)]
    total = np.float32(np.sum(np.asarray(partials, dtype=np.float64)) / B)
    out = np.array(total, dtype=np.float32)
    if _trace:
        return out, res
    return out


# revision 17
# speedup vs baseline: 1.1267x; 1.1267x over previous
"""Trainium2 Bass kernel for nn_Loss_for_localization (YOLO-style loss).

Strategy: pure data parallel over the batch dim (8 images per core), with
host-side dtype compression to cut HBM traffic — the kernel is DMA-bound, so
bytes-on-wire is the whole game:

  objects  -> fp16 (clamped to <=1-2^-11 so log1p(-p) can't hit -inf)
  gt mask  -> uint8 (exact 0/1)
  locs/gt coords -> fp16
  scores/label   -> fp32 (tiny)

Per core the host packs everything into ONE contiguous uint8 tensor
[A=9, 128, 19456]: for each anchor iteration, partition p = b_local*16 + s
holds bytes [p16(2048) | mask(1024) | l16(8192) | g16(8192)], i.e. 1024 cells
of image b's plane (sixteenth s). One dma_start per iteration moves 2.49 MB
with 19456-byte contiguous runs per partition; the HWDGE round-robins
partitions across all SDMA engines. 22.4 MB/core total vs 47.2 MB for fp32.

On-chip per iteration (all elementwise tiles [128, 1024/4096], fp16 so the
DVE runs 2x):
  ACT: lg0 = Ln(1-p) (accum_out -> sum), lg1 = Ln(p)
  DVE: mf = copy(mask u8 -> f16); two stt ops (m * -lg0, m * -lg1, accum);
       dd = l - g  [128,4096]; per channel: md_c = m*d_c, then
       stt md_c*md_c with accum (= m*d^2 since m in {0,1}).
The -100 BCE clamp never binds: fp32-uniform p >= 2^-24 -> |log| <= 16.64,
and the fp16 clamp keeps 1-p >= 2^-11.

Scores cross-entropy + epilogue (weighted combine, cross-partition
ones-matmul reduce) as before. Host sums the 8 partial scalars / B.

This file also carries a workaround for the container's walrus build, which
accepts at most ONE sync-wait and ONE sem-update per instruction: a BIR-JSON
post-pass (hooked into Bass.to_json_bytes) moves excess waits/updates onto
EventSemaphore carrier instructions on the same engine queue.
"""

import sys

sys.path.insert(0, "/opt/trn_rl_repo")

from contextlib import ExitStack

import numpy as np
import orjson

import concourse.bass as bass
import concourse.mybir as mybir
import concourse.tile as tile
from concourse.bass_utils import run_bass_kernel_spmd

f32 = mybir.dt.float32
f16 = mybir.dt.float16
u8 = mybir.dt.uint8
ALU = mybir.AluOpType
ACTF = mybir.ActivationFunctionType
AX = mybir.AxisListType

N_CORES = 8
B, A, H, W, C = 64, 9, 128, 128, 1000
BL = B // N_CORES  # 8 local batches per core
S = 16             # sixteenths of a plane; partition = b_local*16 + s
F = (H * W) // S   # 1024 cells per partition per plane

# packed byte layout per (iter, partition)
OFF_P = 0
OFF_M = OFF_P + 2 * F          # 2048
OFF_L = OFF_M + 2 * F          # 4096
OFF_G = OFF_L + 2 * 4 * F      # 12288
NB = OFF_G + 2 * 4 * F         # 20480

MAXF16_BELOW1 = np.float16(np.nextafter(np.float16(1.0), np.float16(0.0)))

# ---------------------------------------------------------------------------
# walrus <=1 sync-wait / <=1 sem-update per instruction workaround
# ---------------------------------------------------------------------------
_split_counter = [0]


def _carrier(engine, debug, sync_info):
    _split_counter[0] += 1
    return {
        "opcode": "EventSemaphore",
        "engine": engine,
        "ins": [],
        "outs": [],
        "name": f"splitsync-{_split_counter[0]}",
        "debug": debug,
        "sync_info": sync_info,
    }


def _split_excess_sync_json(bir: bytes) -> bytes:
    m = orjson.loads(bir)
    changed = False
    for fn in m.get("functions", []):
        for bb in fn.get("blocks", []):
            instrs = bb.get("instructions")
            if not instrs:
                continue
            out = []
            for ins in instrs:
                si = ins.get("sync_info")
                followers = []
                if si:
                    waits = si.get("on_wait") or []
                    if len(waits) > 1:
                        changed = True
                        for w in waits[:-1]:
                            out.append(
                                _carrier(ins["engine"], ins.get("debug"),
                                         {"on_wait": [w], "on_update": []})
                            )
                        si["on_wait"] = waits[-1:]
                    ups = si.get("on_update") or []
                    if len(ups) > 1:
                        assert ins.get("opcode") != "DMACopy", (
                            "cannot split sem updates off an async DMACopy"
                        )
                        changed = True
                        for u in ups[1:]:
                            followers.append(
                                _carrier(ins["engine"], ins.get("debug"),
                                         {"on_wait": [], "on_update": [u]})
                            )
                        si["on_update"] = ups[:1]
                out.append(ins)
                out.extend(followers)
            bb["instructions"] = out
    if not changed:
        return bir
    return orjson.dumps(m)


if not getattr(bass.Bass, "_sync_split_patched", False):
    _orig_to_json_bytes = bass.Bass.to_json_bytes

    def _patched_to_json_bytes(self):
        return _split_excess_sync_json(_orig_to_json_bytes(self))

    bass.Bass.to_json_bytes = _patched_to_json_bytes
    bass.Bass._sync_split_patched = True


# ---------------------------------------------------------------------------
# program builder
# ---------------------------------------------------------------------------
def _build_program():
    nc = bass.Bass()
    packed_l = nc.declare_dram_parameter("packed", [A, 128, NB], u8, isOutput=False)
    scores_l = nc.declare_dram_parameter("scores", [BL, C], f32, isOutput=False)
    label_l = nc.declare_dram_parameter("label", [BL, 1], f32, isOutput=False)
    out_d = nc.declare_dram_parameter("out", [1, 1], f32, isOutput=True)

    with tile.TileContext(nc) as tc, ExitStack() as octx:
        # long-lived accumulators
        fin = octx.enter_context(tc.tile_pool(name="fin", bufs=1))
        acc_lg0 = fin.tile([128, A], f32)
        acc_msq = fin.tile([128, 2 * A], f32)
        nc.vector.memset(acc_lg0[:], 0.0)
        nc.vector.memset(acc_msq[:], 0.0)


        # ones column for PE column-sum reductions
        ones = fin.tile([128, 1], f16)
        nc.vector.memset(ones[:], 1.0)
        # PSUM accumulators for the masked-log column sums (accumulate
        # across all iterations; read once in the epilogue)
        psacc = octx.enter_context(tc.tile_pool(name="psacc", bufs=1, space="PSUM"))
        ps_p0 = psacc.tile([1, 512], f32)
        ps_p1 = psacc.tile([1, 512], f32)

        # ---- main streaming loop ----
        # ACT's Square runs one iteration behind (software pipelining) so the
        # intra-iteration ACT->DVE->ACT chain never serializes; DVE issues its
        # ACT-independent ops (mask copy, subtract) ahead of the products so
        # its queue head never stalls waiting on Ln.
        with ExitStack() as ctx:
            inp = ctx.enter_context(tc.tile_pool(name="inp", bufs=5))
            mid = ctx.enter_context(tc.tile_pool(name="mid", bufs=2))

            def bcast4(ap):
                # [128, F] AP -> [128, 4, F] with a stride-0 channel dim
                return bass.AP(ap.tensor, ap.offset,
                               [list(ap.ap[0]), [0, 4], list(ap.ap[1])])

            md_prev = None
            for a in range(A):
                big = inp.tile([128, NB], u8, tag="big")
                # small (objects+mask) first so ACT/DVE work on iteration 0
                # starts well before the big coords transfer lands
                nc.sync.dma_start(big[:, :OFF_L], packed_l[a][:, :OFF_L])
                nc.sync.dma_start(big[:, OFF_L:], packed_l[a][:, OFF_L:])

                p16 = big[:, OFF_P:OFF_M].bitcast(f16)   # [128,1024]
                mfv = big[:, OFF_M:OFF_L].bitcast(f16)   # [128,1024] mask
                l16 = big[:, OFF_L:OFF_G].bitcast(f16)   # [128,4096]
                g16 = big[:, OFF_G:NB].bitcast(f16)      # [128,4096]

                lg0 = mid.tile([128, F], f16, tag="lg0")
                nc.scalar.activation(lg0[:], p16, ACTF.Ln, bias=1.0, scale=-1.0,
                                     accum_out=acc_lg0[:, a:a + 1])
                lg1 = mid.tile([128, F], f16, tag="lg1")
                nc.scalar.activation(lg1[:], p16, ACTF.Ln)
                if md_prev is not None:
                    sq = mid.tile([128, 4 * F], f16, tag="sq", bufs=1)
                    for h in range(2):
                        hs = slice(2 * F * h, 2 * F * (h + 1))
                        nc.scalar.activation(
                            sq[:, hs], md_prev[:, hs], ACTF.Square,
                            accum_out=acc_msq[:, 2 * (a - 1) + h:2 * (a - 1) + h + 1])

                dd = mid.tile([128, 4 * F], f16, tag="dd")
                nc.vector.tensor_tensor(out=dd[:], in0=l16, in1=g16,
                                        op=ALU.subtract)

                # md = m * d per channel (2x tt); m^2 = m, so Square(md)
                # with accum_out on ACT gives sum(m * d^2) for free.
                # Issued before pr0/pr1 so the DVE queue head never waits
                # on ACT's Ln outputs.
                md = mid.tile([128, 4 * F], f16, tag="md")
                for c in range(4):
                    nc.vector.tensor_tensor(
                        out=md[:, c * F:(c + 1) * F],
                        in0=dd[:, c * F:(c + 1) * F], in1=mfv, op=ALU.mult)
                md_prev = md

                # masked log sums: tt product (2x), then PE column-sum
                # accumulated into ps_p0/ps_p1 across all iterations
                pr0 = mid.tile([128, F], f16, tag="pr0")
                nc.vector.tensor_tensor(out=pr0[:], in0=mfv, in1=lg0[:],
                                        op=ALU.mult)
                pr1 = mid.tile([128, F], f16, tag="pr1")
                nc.vector.tensor_tensor(out=pr1[:], in0=mfv, in1=lg1[:],
                                        op=ALU.mult)
                for h in range(2):
                    sl = slice(512 * h, 512 * (h + 1))
                    nc.tensor.matmul(ps_p0[:], ones[:], pr0[:, sl],
                                     start=(a == 0 and h == 0),
                                     stop=(a == A - 1 and h == 1))
                    nc.tensor.matmul(ps_p1[:], ones[:], pr1[:, sl],
                                     start=(a == 0 and h == 0),
                                     stop=(a == A - 1 and h == 1))

            sq = mid.tile([128, 4 * F], f16, tag="sq", bufs=1)
            for h in range(2):
                hs = slice(2 * F * h, 2 * F * (h + 1))
                nc.scalar.activation(
                    sq[:, hs], md_prev[:, hs], ACTF.Square,
                    accum_out=acc_msq[:, 2 * (A - 1) + h:2 * (A - 1) + h + 1])

        # ---- image-classification cross entropy ----
        img = fin.tile([BL, 1], f32)
        sco_ctx = ExitStack()
        sco = sco_ctx.enter_context(tc.tile_pool(name="sco", bufs=1))
        sct = sco.tile([BL, C], f32)
        nc.sync.dma_start(sct[:], scores_l[:])
        labt = sco.tile([BL, 1], f32)
        nc.sync.dma_start(labt[:], label_l[:])

        mx = sco.tile([BL, 1], f32)
        nc.vector.tensor_reduce(out=mx[:], in_=sct[:], axis=AX.X, op=ALU.max)
        nmx = sco.tile([BL, 1], f32)
        nc.vector.tensor_scalar(out=nmx[:], in0=mx[:], scalar1=-1.0,
                                scalar2=None, op0=ALU.mult)
        et = sco.tile([BL, C], f32)
        se = sco.tile([BL, 1], f32)
        nc.scalar.activation(et[:], sct[:], ACTF.Exp, bias=nmx[:], scale=1.0,
                             accum_out=se[:])
        lse0 = sco.tile([BL, 1], f32)
        nc.scalar.activation(lse0[:], se[:], ACTF.Ln)
        lse = sco.tile([BL, 1], f32)
        nc.vector.tensor_tensor(out=lse[:], in0=lse0[:], in1=mx[:], op=ALU.add)

        io = sco.tile([BL, C], mybir.dt.int32)
        nc.gpsimd.iota(io[:], pattern=[[1, C]], base=0, channel_multiplier=0)
        iof = sco.tile([BL, C], f32)
        nc.vector.tensor_copy(iof[:], io[:])
        oh = sco.tile([BL, C], f32)
        nc.vector.tensor_scalar(out=oh[:], in0=iof[:], scalar1=labt[:],
                                scalar2=None, op0=ALU.is_equal)
        jk2 = sco.tile([BL, C], f32)
        pk = sco.tile([BL, 1], f32)
        nc.vector.scalar_tensor_tensor(out=jk2[:], in0=oh[:], scalar=1.0,
                                       in1=sct[:], op0=ALU.mult, op1=ALU.mult,
                                       accum_out=pk[:])
        nc.vector.scalar_tensor_tensor(out=img[:], in0=pk[:], scalar=-1.0,
                                       in1=lse[:], op0=ALU.mult, op1=ALU.add)
        sco_ctx.close()

        # ---- per-partition reductions & weighted combine ----
        # total = img_sum - 0.5*sum(acc_lg0) + 0.5*sum(ps_p0) - sum(ps_p1)
        #         + 5*sum(acc_msq)
        with ExitStack() as ectx:
            epi = ectx.enter_context(tc.tile_pool(name="epi", bufs=1))
            psum = ectx.enter_context(tc.tile_pool(name="psum", bufs=1, space="PSUM"))

            s_lg0 = epi.tile([128, 1], f32)
            s_msq = epi.tile([128, 1], f32)
            nc.vector.tensor_reduce(out=s_lg0[:], in_=acc_lg0[:], axis=AX.X, op=ALU.add)
            nc.vector.tensor_reduce(out=s_msq[:], in_=acc_msq[:], axis=AX.X, op=ALU.add)

            tB = epi.tile([128, 1], f32)
            nc.vector.tensor_scalar(out=tB[:], in0=s_lg0[:], scalar1=-0.5,
                                    scalar2=None, op0=ALU.mult)
            tvec = epi.tile([128, 1], f32)
            nc.vector.scalar_tensor_tensor(out=tvec[:], in0=s_msq[:], scalar=5.0,
                                           in1=tB[:], op0=ALU.mult, op1=ALU.add)

            # ---- cross-partition reduce via ones-matmul into PSUM ----
            ones32 = epi.tile([128, 1], f32)
            nc.vector.memset(ones32[:], 1.0)
            acc = psum.tile([1, 1], f32)
            nc.tensor.matmul(acc[:], ones32[:], tvec[:], start=True, stop=False)
            nc.tensor.matmul(acc[:], ones32[:BL, :], img[:], start=False, stop=True)

            # fold in the PSUM column-sum accumulators
            jk0 = epi.tile([1, 512], f32)
            s0 = epi.tile([1, 1], f32)
            nc.scalar.activation(jk0[:], ps_p0[:], ACTF.Copy, accum_out=s0[:])
            jk1 = epi.tile([1, 512], f32)
            s1 = epi.tile([1, 1], f32)
            nc.scalar.activation(jk1[:], ps_p1[:], ACTF.Copy, accum_out=s1[:])

            accs = epi.tile([1, 1], f32)
            nc.scalar.copy(out=accs[:], in_=acc[:])
            f1 = epi.tile([1, 1], f32)
            nc.vector.scalar_tensor_tensor(out=f1[:], in0=s0[:], scalar=0.5,
                                           in1=accs[:], op0=ALU.mult, op1=ALU.add)
            res = epi.tile([1, 1], f32)
            nc.vector.scalar_tensor_tensor(out=res[:], in0=s1[:], scalar=-1.0,
                                           in1=f1[:], op0=ALU.mult, op1=ALU.add)
            nc.sync.dma_start(out_d[:], res[:])

    return nc


_program_cache = {}


def _get_program():
    if "nc" not in _program_cache:
        _program_cache["nc"] = _build_program()
    return _program_cache["nc"]


def _pack_core(objects, locs, gt):
    """Per-core [BL,...] inputs -> packed u8 [A, 128, NB].

    Partition p = b*16 + s; each partition's F cells are the s-th sixteenth
    of image b's (anchor-a) plane, flattened h*W+w order.
    """
    p16 = np.minimum(objects.astype(np.float16), MAXF16_BELOW1)
    p16 = p16.reshape(BL, A, S, F).transpose(1, 0, 2, 3)       # [A,BL,S,F]
    p16 = np.ascontiguousarray(p16).reshape(A, 128, F)
    m16 = gt[:, :, 0].astype(np.float16)
    m16 = m16.reshape(BL, A, S, F).transpose(1, 0, 2, 3)
    m16 = np.ascontiguousarray(m16).reshape(A, 128, F)
    l16 = locs.astype(np.float16).reshape(BL, A, 4, S, F).transpose(1, 0, 3, 2, 4)
    l16 = np.ascontiguousarray(l16).reshape(A, 128, 4 * F)     # [A,128,4096]
    g16 = gt[:, :, 1:5].astype(np.float16).reshape(BL, A, 4, S, F)
    g16 = np.ascontiguousarray(g16.transpose(1, 0, 3, 2, 4)).reshape(A, 128, 4 * F)
    return np.concatenate(
        [p16.view(np.uint8), m16.view(np.uint8), l16.view(np.uint8), g16.view(np.uint8)], axis=2)


def kernel(objects, scores, locs, label, gt, _trace=False, _trace_kwargs=None):
    objects = np.asarray(objects, dtype=np.float32)
    scores = np.ascontiguousarray(np.asarray(scores, dtype=np.float32))
    locs = np.asarray(locs, dtype=np.float32)
    gt = np.asarray(gt, dtype=np.float32)
    labf = np.asarray(label).astype(np.float32).reshape(B, 1)

    nc = _get_program()
    in_maps = []
    for i in range(N_CORES):
        sl = slice(i * BL, (i + 1) * BL)
        in_maps.append({
            "packed": _pack_core(objects[sl], locs[sl], gt[sl]),
            "scores": scores[sl],
            "label": np.ascontiguousarray(labf[sl]),
        })

    kw = {}
    if _trace:
        kw["trace"] = True
        kw.update(_trace_kwargs or {})
    res = run_bass_kernel_spmd(nc, in_maps, list(range(N_CORES)), **kw)
    partials = [float(res.results[i]["out"][0, 0]) for i in range(N_CORES)]
    total = np.float32(np.sum(np.asarray(partials, dtype=np.float64)) / B)
    out = np.array(total, dtype=np.float32)
    if _trace:
        return out, res
    return out
